# revision 13
# baseline (speedup 1.0000x reference)
"""Trainium2 Bass kernel for the HNEPY GNN message-passing problem.

Strategy (8 NeuronCores, SPMD):
  - Row-shard A across cores as host-transposed shards At_i = A[rows_i,:].T
    ([N, R] contiguous), so the TensorE contraction axis (A columns) lands on
    SBUF partitions.
  - Each core encodes its 1/8 slice of each node-type feature table
    (transposed on host), transposes the [16, rows] result back to natural
    layout on the TensorEngine, and AllGathers X per table (natural order).
  - A@X computed transposed: Y^T[16, R] += X_tile[128,16].T @ At_tile[128, R],
    PSUM-accumulated over 110 k-tiles while At streams from HBM (memory
    bound: 98MB/core).
  - MLP + bilinear tables computed in transposed form, packed into a 64-col
    gather table G = [emb | emb@B1 | emb@B2m | emb@W_B2/3 + (b_B2+b_lin)/3],
    transposed to natural layout, AllGathered.
  - Edge scoring: dma_gather 3 roles x 2 polarities (12544 edges/core each),
    per-edge 16-dots on VectorE, tanh on ScalarE. Outputs per-edge tanh
    triples; host applies the tiny W_sim combination and the final loss.
"""
import sys

sys.path.insert(0, "/opt/trn_rl_repo")
import numpy as np
import ml_dtypes
import os

import concourse.bacc as bacc
import concourse.mybir as mybir
import concourse.tile as tile
from concourse import masks
from concourse.bass_utils import run_bass_kernel_spmd

NCORES = 8
N1, N2, N3 = 4000, 6000, 4000
N = N1 + N2 + N3  # 14000
R = N // NCORES  # 1750 A-rows per core
E = 100000
EC = E // NCORES  # 12500 edges per core per polarity
ECP = 12544  # padded to a multiple of 128
GRP = ECP // 128  # 98
R1, R2, R3 = 16, 32, 16
D1, D2, D3 = 1024, 512, 256
S1, S2, S3 = N1 // NCORES, N2 // NCORES, N3 // NCORES  # 500, 750, 500
GW = 64  # gather table row width in f32 (256B, dma_gather minimum)
F32 = mybir.dt.float32
I16 = mybir.dt.int16
AF = mybir.ActivationFunctionType
ALU = mybir.AluOpType
AX = mybir.AxisListType

KT = [(t, min(128, N - t)) for t in range(0, N, 128)]  # contraction tiles
NB = [(s, min(512, R - s)) for s in range(0, R, 512)]  # output row blocks

BF16_A = os.environ.get("K_BF16", "1") == "1"
_CACHE = {}


class _StageDone(Exception):
    pass


def _build(dbg=False, stage=4):
    key = ("nc", dbg, stage)
    if key in _CACHE:
        return _CACHE[key]
    nc = bacc.Bacc("TRN2", target_bir_lowering=False, debug=False, num_devices=NCORES)

    din = lambda name, shape, dt=F32: nc.dram_tensor(name, shape, dt, kind="ExternalInput")
    BF16 = mybir.dt.bfloat16
    at = din("at", [N, R], BF16 if BF16_A else F32)
    d1t, d2t, d3t = din("d1t", [D1, S1]), din("d2t", [D2, S2]), din("d3t", [D3, S3])
    we1, we2, we3 = din("we1", [D1, R1]), din("we2", [D2, R1]), din("we3", [D3, R1])
    ebt = din("ebt", [R1, 3])
    wg1, bg1c = din("wg1", [R1, R2]), din("bg1c", [R2, 1])
    wg2, bg2c = din("wg2", [R2, R3]), din("bg2c", [R3, 1])
    b1m, b2m = din("b1m", [R3, R3]), din("b2m", [R3, R3])
    wb2s, b3c = din("wb2s", [R3, 3]), din("b3c", [3, 1])
    eidx = din("eidx", [128, 6, ECP // 16], I16)

    tout = nc.dram_tensor("tout", [128, 6, GRP], F32, kind="ExternalOutput")
    if dbg:
        dbg_gd = nc.dram_tensor("dbg_gd", [128, GRP, GW], F32, kind="ExternalOutput")
        dbg_x = nc.dram_tensor("dbg_x", [128, len(KT) * R1], F32, kind="ExternalOutput")
        dbg_y = nc.dram_tensor("dbg_y", [R1, R], F32, kind="ExternalOutput")
        dbg_emb = nc.dram_tensor("dbg_emb", [R3, R], F32, kind="ExternalOutput")
        dbg_g = nc.dram_tensor("dbg_g", [R, GW], F32, kind="ExternalOutput")

    e1b = nc.dram_tensor("e1b", [S1, R1], F32)
    e2b = nc.dram_tensor("e2b", [S2, R1], F32)
    e3b = nc.dram_tensor("e3b", [S3, R1], F32)
    x1 = nc.dram_tensor("x1", [N1, R1], F32, addr_space="Shared")
    x2 = nc.dram_tensor("x2", [N2, R1], F32, addr_space="Shared")
    x3 = nc.dram_tensor("x3", [N3, R1], F32, addr_space="Shared")
    gb = nc.dram_tensor("gb", [R, GW], F32)
    gall = nc.dram_tensor("gall", [N, GW], F32, addr_space="Shared")

    rgroups = [list(range(NCORES))]

    with tile.TileContext(nc) as tc:
        with (
            tc.tile_pool(name="const", bufs=1) as constp,
            tc.tile_pool(name="feat", bufs=1) as featp,
            tc.tile_pool(name="arhs", bufs=3) as arhsp,
            tc.tile_pool(name="small", bufs=1) as smallp,
            tc.tile_pool(name="gath", bufs=1) as gathp,
            tc.tile_pool(name="sc", bufs=1) as scp,
            tc.tile_pool(name="psY", bufs=4, space="PSUM") as psY,
            tc.tile_pool(name="psA", bufs=2, space="PSUM") as psA,
            tc.tile_pool(name="psB", bufs=2, space="PSUM") as psB,
        ):
          def _phases():
            ident = constp.tile([128, 128], F32)
            masks.make_identity(nc, ident[:])

            def cload(name, shape):
                t = constp.tile(shape, F32, tag=name)
                nc.sync.dma_start(t[:], globals_map[name][tuple(slice(None) for _ in shape)])
                return t

            globals_map = dict(ebt=ebt, wg1=wg1, bg1c=bg1c, wg2=wg2, bg2c=bg2c,
                               b1m=b1m, b2m=b2m, wb2s=wb2s, b3c=b3c)
            ebt_sb = cload("ebt", [R1, 3])
            wg1_sb = cload("wg1", [R1, R2])
            bg1_sb = cload("bg1c", [R2, 1])
            wg2_sb = cload("wg2", [R2, R3])
            bg2_sb = cload("bg2c", [R3, 1])
            b1m_sb = cload("b1m", [R3, R3])
            b2m_sb = cload("b2m", [R3, R3])
            wb2s_sb = cload("wb2s", [R3, 3])
            b3_sb = cload("b3c", [3, 1])

            # encoder weights: [D, 16] -> sbuf [128, D/128, 16]
            enc_w = []
            for nm, wd, D in (("we1", we1, D1), ("we2", we2, D2), ("we3", we3, D3)):
                t = constp.tile([128, D // 128, R1], F32, tag=nm)
                nc.sync.dma_start(t[:], wd.ap().rearrange("(t p) f -> p t f", p=128))
                enc_w.append(t)

            eidx_sb = constp.tile([128, 6, ECP // 16], I16, tag="eidx")
            nc.sync.dma_start(eidx_sb[:], eidx[:, :, :])

            # ---------------- encoders: xcat[16, 1750] = [e1^T | e2^T | e3^T]
            xcat = smallp.tile([R1, R], F32, tag="xcat")
            enc_cfg = [
                (d1t, enc_w[0], 0, D1, S1, 0),
                (d2t, enc_w[1], 1, D2, S2, S1),
                (d3t, enc_w[2], 2, D3, S3, S1 + S2),
            ]
            for fd, w_sb, bcol, D, S, xoff in enc_cfg:
                nkt = D // 128
                ft = featp.tile([128, nkt, S], F32, tag="feat", name=f"feat{bcol}")
                nc.sync.dma_start(ft[:], fd.ap().rearrange("(t p) s -> p t s", p=128))
                for ns in range(0, S, 512):
                    nw = min(512, S - ns)
                    ps = psA.tile([R1, 512], F32, tag="psa")
                    for t in range(nkt):
                        nc.tensor.matmul(
                            ps[:R1, :nw], w_sb[:, t, :], ft[:, t, ns:ns + nw],
                            start=(t == 0), stop=(t == nkt - 1),
                        )
                    nc.scalar.activation(
                        xcat[:, xoff + ns:xoff + ns + nw], ps[:R1, :nw],
                        AF.Tanh, bias=ebt_sb[:, bcol:bcol + 1],
                    )

            # transpose xcat to natural-order bounce buffers
            for src_off, S, bdram in ((0, S1, e1b), (S1, S2, e2b), (S1 + S2, S3, e3b)):
                for c0 in range(0, S, 128):
                    cw = min(128, S - c0)
                    pt = psB.tile([128, 512], F32, tag="psb")
                    nc.tensor.matmul(
                        pt[:cw, :R1], xcat[:R1, src_off + c0:src_off + c0 + cw],
                        ident[:R1, :R1], is_transpose=True,
                    )
                    st = scp.tile([128, R1], F32, tag="tstage")
                    nc.vector.tensor_copy(st[:cw, :], pt[:cw, :R1])
                    nc.sync.dma_start(bdram[c0:c0 + cw, :], st[:cw, :])

            for bdram, xdram in ((e1b, x1), (e2b, x2), (e3b, x3)):
                nc.gpsimd.collective_compute(
                    "AllGather", ALU.bypass, replica_groups=rgroups,
                    ins=[bdram[:, :]], outs=[xdram[:, :]],
                )

            # load full X (in A-column order) into SBUF: [128, 110, 16]
            xall = smallp.tile([128, len(KT), R1], F32, tag="xall")

            def xsrc(g):
                if g < N1:
                    return x1, g, N1
                if g < N1 + N2:
                    return x2, g - N1, N1 + N2
                return x3, g - N1 - N2, N

            for ti, (t0, tk) in enumerate(KT):
                g = t0
                while g < t0 + tk:
                    dram, loc, lim = xsrc(g)
                    seg = min(t0 + tk, lim) - g
                    nc.sync.dma_start(
                        xall[g - t0:g - t0 + seg, ti, :], dram[loc:loc + seg, :]
                    )
                    g += seg

            if dbg:
                nc.sync.dma_start(dbg_x[:, :], xall[:].rearrange("p t f -> p (t f)"))
            if stage < 2:
                return
            # ---------------- main A@X: Y^T[16, 1750], PSUM-accumulated
            adt = BF16 if BF16_A else F32
            if BF16_A:
                xmm = smallp.tile([128, len(KT), R1], BF16, tag="xbf")
                nc.vector.tensor_copy(xmm[:], xall[:])
            else:
                xmm = xall
            psy = [psY.tile([R1, 512], F32, tag="psy", name=f"psy{i}")
                   for i in range(len(NB))]
            for ti, (t0, tk) in enumerate(KT):
                rt = arhsp.tile([128, R], adt, tag="arhs")
                nc.sync.dma_start(rt[:tk, :], at[t0:t0 + tk, :])
                for nbi, (ns, nw) in enumerate(NB):
                    nc.tensor.matmul(
                        psy[nbi][:R1, :nw], xmm[:tk, ti, :], rt[:tk, ns:ns + nw],
                        start=(ti == 0), stop=(ti == len(KT) - 1),
                    )
            ysb = smallp.tile([R1, R], F32, tag="ysb")
            for nbi, (ns, nw) in enumerate(NB):
                nc.scalar.copy(ysb[:, ns:ns + nw], psy[nbi][:R1, :nw])
            if dbg:
                nc.sync.dma_start(dbg_y[:, :], ysb[:])

            if stage < 3:
                return
            # ---------------- MLP + gather-table build (all transposed)
            hsb = smallp.tile([R2, R], F32, tag="hsb")
            for ns, nw in NB:
                ph = psB.tile([R2, 512], F32, tag="psb")
                nc.tensor.matmul(ph[:R2, :nw], wg1_sb[:R1, :R2], ysb[:R1, ns:ns + nw],
                                 start=True, stop=True)
                nc.scalar.activation(hsb[:R2, ns:ns + nw], ph[:R2, :nw], AF.Tanh,
                                     bias=bg1_sb[:, 0:1])
            # table bands at 32-aligned partition starts (compute-engine APs
            # must start at partition 0/32/64/96): emb@0, T1@32, T2@64, TW@96
            S_sb = smallp.tile([128, R], F32, tag="stab")
            for ns, nw in NB:
                pe = psB.tile([R3, 512], F32, tag="psb")
                nc.tensor.matmul(pe[:R3, :nw], wg2_sb[:R2, :R3], hsb[:R2, ns:ns + nw],
                                 start=True, stop=True)
                nc.scalar.activation(S_sb[0:R3, ns:ns + nw], pe[:R3, :nw], AF.Identity,
                                     bias=bg2_sb[:, 0:1])
            if dbg:
                nc.sync.dma_start(dbg_emb[:, :], S_sb[0:R3, :])
            for ns, nw in NB:
                p1 = psB.tile([R3, 512], F32, tag="psb")
                nc.tensor.matmul(p1[:R3, :nw], b1m_sb[:R3, :R3], S_sb[0:R3, ns:ns + nw],
                                 start=True, stop=True)
                nc.scalar.copy(S_sb[32:48, ns:ns + nw], p1[:R3, :nw])
                p2 = psB.tile([R3, 512], F32, tag="psb")
                nc.tensor.matmul(p2[:R3, :nw], b2m_sb[:R3, :R3], S_sb[0:R3, ns:ns + nw],
                                 start=True, stop=True)
                nc.scalar.copy(S_sb[64:80, ns:ns + nw], p2[:R3, :nw])
                pw = psB.tile([3, 512], F32, tag="psb")
                nc.tensor.matmul(pw[:3, :nw], wb2s_sb[:R3, :3], S_sb[0:R3, ns:ns + nw],
                                 start=True, stop=True)
                nc.scalar.activation(S_sb[96:99, ns:ns + nw], pw[:3, :nw], AF.Identity,
                                     bias=b3_sb[:, 0:1])

            # transpose S -> compact 64-col rows -> gb [1750, 64] -> AllGather
            # (cols 51:64 of gb are unwritten garbage; never read in compute)
            for c0 in range(0, R, 128):
                cw = min(128, R - c0)
                pg = psB.tile([128, 512], F32, tag="psb")
                nc.tensor.matmul(pg[:cw, :128], S_sb[:, c0:c0 + cw],
                                 ident[:, :128], is_transpose=True)
                sg = scp.tile([128, GW], F32, tag="gstage")
                nc.vector.tensor_copy(
                    sg[:cw, :].rearrange("p (g c) -> p g c", c=16),
                    pg[:cw, 0:128].rearrange("p (g c) -> p g c", c=32)[:, :, 0:16],
                )
                nc.sync.dma_start(gb[c0:c0 + cw, :], sg[:cw, :])
            nc.gpsimd.collective_compute(
                "AllGather", ALU.bypass, replica_groups=rgroups,
                ins=[gb[:, :]], outs=[gall[:, :]],
            )
            if dbg:
                nc.sync.dma_start(dbg_g[:, :], gb[:, :])

            if stage < 4:
                return
            # ---------------- edge scoring
            if stage == 35:
                import os
                gch = int(os.environ.get("K_GCHUNK", str(ECP)))
                gd0 = gathp.tile([128, GRP, GW], F32, tag="gd")
                for c0 in range(0, ECP, gch):
                    cn = min(gch, ECP - c0)
                    nc.gpsimd.dma_gather(
                        gd0[:, c0 // 128:(c0 + cn) // 128, :], gall[:, :],
                        eidx_sb[:, 0, c0 // 16:(c0 + cn) // 16],
                        num_idxs=cn, num_idxs_reg=cn, elem_size=GW,
                    )
                if dbg:
                    nc.sync.dma_start(dbg_gd[:, :, :], gd0[:])
                return
            tsb = smallp.tile([128, 6, GRP], F32, tag="tsb")
            for pol in range(2):
                gd = gathp.tile([128, GRP, GW], F32, tag="gd")
                gi = gathp.tile([128, GRP, GW], F32, tag="gi")
                ga = gathp.tile([128, GRP, GW], F32, tag="ga")
                for t, j in ((gd, 3 * pol), (gi, 3 * pol + 1), (ga, 3 * pol + 2)):
                    for c0 in range(0, ECP, 1024):
                        cn = min(1024, ECP - c0)
                        nc.gpsimd.dma_gather(
                            t[:, c0 // 128:(c0 + cn) // 128, :], gall[:, :],
                            eidx_sb[:, j, c0 // 16:(c0 + cn) // 16],
                            num_idxs=cn, num_idxs_reg=cn, elem_size=GW,
                        )
                prod = scp.tile([128, GRP, R3], F32, tag="prod")
                b1 = scp.tile([128, GRP], F32, tag="b1")
                nc.vector.tensor_tensor(prod[:], gd[:, :, 16:32], gi[:, :, 0:16], op=ALU.mult)
                nc.vector.tensor_reduce(b1[:], prod[:], axis=AX.X, op=ALU.add)
                prod2 = scp.tile([128, GRP, R3], F32, tag="prod2")
                b2 = scp.tile([128, GRP], F32, tag="b2")
                nc.vector.tensor_tensor(prod2[:], gd[:, :, 32:48], ga[:, :, 0:16], op=ALU.mult)
                nc.vector.tensor_reduce(b2[:], prod2[:], axis=AX.X, op=ALU.add)
                vt = scp.tile([128, GRP, 3], F32, tag="vt")
                v = scp.tile([128, GRP, 3], F32, tag="v")
                nc.vector.tensor_tensor(vt[:], gd[:, :, 48:51], gi[:, :, 48:51], op=ALU.add)
                nc.vector.tensor_tensor(v[:], vt[:], ga[:, :, 48:51], op=ALU.add)
                a1 = scp.tile([128, GRP], F32, tag="a1")
                a2 = scp.tile([128, GRP], F32, tag="a2")
                nc.vector.tensor_tensor(a1[:], b1[:], v[:, :, 0], op=ALU.add)
                nc.vector.tensor_tensor(a2[:], b2[:], v[:, :, 1], op=ALU.add)
                nc.scalar.activation(tsb[:, 3 * pol + 0, :], a1[:], AF.Tanh)
                nc.scalar.activation(tsb[:, 3 * pol + 1, :], a2[:], AF.Tanh)
                nc.scalar.activation(tsb[:, 3 * pol + 2, :], v[:, :, 2], AF.Tanh)
            nc.sync.dma_start(tout[:, :, :], tsb[:])

          _phases()

    nc.compile()
    _CACHE[key] = nc
    return nc


def _wrap_idx(ids):
    """dma_gather index layout: [128, n/16] int16, 16-partition wrap x8 replicas."""
    assert ids.shape[0] == ECP
    w = ids.astype(np.int16).reshape(ECP // 16, 16).T  # [16, n/16]
    return np.tile(w, (8, 1)).copy()


def _prep_inputs(inputs):
    A = np.asarray(inputs["A"], np.float32)
    d1, d2, d3 = (np.asarray(inputs[k], np.float32) for k in ("d1_fea", "d2_fea", "d3_fea"))
    f32 = lambda k: np.ascontiguousarray(np.asarray(inputs[k], np.float32))
    shared = {
        "we1": f32("W_e1"), "we2": f32("W_e2"), "we3": f32("W_e3"),
        "ebt": np.stack([f32("b_e1"), f32("b_e2"), f32("b_e3")], axis=1),
        "wg1": f32("Wg1"), "bg1c": f32("bg1")[:, None],
        "wg2": f32("Wg2"), "bg2c": f32("bg2")[:, None],
        "b1m": f32("B1"), "b2m": f32("B2m"),
        "wb2s": f32("W_B2") / np.float32(3.0),
        "b3c": ((f32("b_B2") + f32("b_lin")) / np.float32(3.0))[:, None],
    }
    pos = np.asarray(inputs["pos_edges"])
    neg = np.asarray(inputs["neg_edges"])
    offs = np.array([0, N1, 6000], np.int32)  # drug, indi, adr(bugged d3_eb slice)
    in_maps = []
    for c in range(NCORES):
        m = dict(shared)
        r0 = c * R
        m["at"] = np.ascontiguousarray(A[r0:r0 + R, :].T)
        if BF16_A:
            m["at"] = m["at"].astype(ml_dtypes.bfloat16)
        m["d1t"] = np.ascontiguousarray(d1[c * S1:(c + 1) * S1].T)
        m["d2t"] = np.ascontiguousarray(d2[c * S2:(c + 1) * S2].T)
        m["d3t"] = np.ascontiguousarray(d3[c * S3:(c + 1) * S3].T)
        eidx = np.zeros((128, 6, ECP // 16), np.int16)
        for pol, edges in enumerate((pos, neg)):
            sl = edges[c * EC:(c + 1) * EC]
            for role in range(3):
                ids = np.zeros(ECP, np.int32)
                ids[:EC] = sl[:, role, 1].astype(np.int32) + offs[role]
                eidx[:, 3 * pol + role, :] = _wrap_idx(ids)
        m["eidx"] = eidx
        in_maps.append(m)
    return in_maps


def _finish(results, inputs):
    wsim = np.asarray(inputs["W_sim"], np.float32)[:, 0]
    bsim = np.asarray(inputs["b_sim"], np.float32)[0]
    parts = []
    for c in range(NCORES):
        arr = results[c]["tout"]  # [128, 6, 98]; edge g*128+p at [p, j, g]
        parts.append(arr.transpose(1, 2, 0).reshape(6, ECP)[:, :EC])
    T = np.concatenate(parts, axis=1).astype(np.float32)  # [6, 100000]
    Se = wsim[0] * T[0] + wsim[1] * T[1] + wsim[2] * T[2] + bsim
    Se0 = wsim[0] * T[3] + wsim[1] * T[4] + wsim[2] * T[5] + bsim
    m0 = np.float32(Se0.mean())
    loss = np.log1p(np.exp(m0 - Se)).mean()
    return np.asarray(loss, dtype=np.float32)


def run(inputs, trace=False, dbg=False):
    nc = _build(dbg=dbg)
    in_maps = _prep_inputs(inputs)
    res = run_bass_kernel_spmd(nc, in_maps, list(range(NCORES)), trace=trace)
    return res


def kernel(**inputs) -> np.ndarray:
    res = run(inputs)
    return _finish(res.results, inputs)


# revision 15
# speedup vs baseline: 1.0114x; 1.0114x over previous
"""Trainium2 Bass kernel for the HNEPY GNN message-passing problem.

Strategy (8 NeuronCores, SPMD):
  - Row-shard A across cores as host-transposed shards At_i = A[rows_i,:].T
    ([N, R] contiguous), so the TensorE contraction axis (A columns) lands on
    SBUF partitions.
  - Each core encodes its 1/8 slice of each node-type feature table
    (transposed on host), transposes the [16, rows] result back to natural
    layout on the TensorEngine, and AllGathers X per table (natural order).
  - A@X computed transposed: Y^T[16, R] += X_tile[128,16].T @ At_tile[128, R],
    PSUM-accumulated over 110 k-tiles while At streams from HBM (memory
    bound: 98MB/core).
  - MLP + bilinear tables computed in transposed form, packed into a 64-col
    gather table G = [emb | emb@B1 | emb@B2m | emb@W_B2/3 + (b_B2+b_lin)/3],
    transposed to natural layout, AllGathered.
  - Edge scoring: dma_gather 3 roles x 2 polarities (12544 edges/core each),
    per-edge 16-dots on VectorE, tanh on ScalarE. Outputs per-edge tanh
    triples; host applies the tiny W_sim combination and the final loss.
"""
import sys

sys.path.insert(0, "/opt/trn_rl_repo")
import numpy as np
import ml_dtypes
import os

import concourse.bacc as bacc
import concourse.mybir as mybir
import concourse.tile as tile
from concourse import masks
from concourse.bass_utils import run_bass_kernel_spmd

NCORES = 8
N1, N2, N3 = 4000, 6000, 4000
N = N1 + N2 + N3  # 14000
R = N // NCORES  # 1750 A-rows per core
E = 100000
EC = E // NCORES  # 12500 edges per core per polarity
ECP = 12544  # padded to a multiple of 128
GRP = ECP // 128  # 98
R1, R2, R3 = 16, 32, 16
D1, D2, D3 = 1024, 512, 256
S1, S2, S3 = N1 // NCORES, N2 // NCORES, N3 // NCORES  # 500, 750, 500
GW = 64  # gather table row width in f32 (256B, dma_gather minimum)
F32 = mybir.dt.float32
I16 = mybir.dt.int16
AF = mybir.ActivationFunctionType
ALU = mybir.AluOpType
AX = mybir.AxisListType

KT = [(t, min(128, N - t)) for t in range(0, N, 128)]  # contraction tiles
NB = [(s, min(512, R - s)) for s in range(0, R, 512)]  # output row blocks

BF16_A = os.environ.get("K_BF16", "1") == "1"
_CACHE = {}


class _StageDone(Exception):
    pass


def _build(dbg=False, stage=4):
    key = ("nc", dbg, stage)
    if key in _CACHE:
        return _CACHE[key]
    nc = bacc.Bacc("TRN2", target_bir_lowering=False, debug=False, num_devices=NCORES)

    din = lambda name, shape, dt=F32: nc.dram_tensor(name, shape, dt, kind="ExternalInput")
    BF16 = mybir.dt.bfloat16
    at = din("at", [N, R], BF16 if BF16_A else F32)
    d1t, d2t, d3t = din("d1t", [D1, S1]), din("d2t", [D2, S2]), din("d3t", [D3, S3])
    we1, we2, we3 = din("we1", [D1, R1]), din("we2", [D2, R1]), din("we3", [D3, R1])
    ebt = din("ebt", [R1, 3])
    wg1, bg1c = din("wg1", [R1, R2]), din("bg1c", [R2, 1])
    wg2, bg2c = din("wg2", [R2, R3]), din("bg2c", [R3, 1])
    b1m, b2m = din("b1m", [R3, R3]), din("b2m", [R3, R3])
    wb2s, b3c = din("wb2s", [R3, 3]), din("b3c", [3, 1])
    eidx = din("eidx", [128, 6, ECP // 16], I16)

    tout = nc.dram_tensor("tout", [128, 6, GRP], F32, kind="ExternalOutput")
    if dbg:
        dbg_gd = nc.dram_tensor("dbg_gd", [128, GRP, GW], F32, kind="ExternalOutput")
        dbg_x = nc.dram_tensor("dbg_x", [128, len(KT) * R1], F32, kind="ExternalOutput")
        dbg_y = nc.dram_tensor("dbg_y", [R1, R], F32, kind="ExternalOutput")
        dbg_emb = nc.dram_tensor("dbg_emb", [R3, R], F32, kind="ExternalOutput")
        dbg_g = nc.dram_tensor("dbg_g", [R, GW], F32, kind="ExternalOutput")

    e1b = nc.dram_tensor("e1b", [S1, R1], F32)
    e2b = nc.dram_tensor("e2b", [S2, R1], F32)
    e3b = nc.dram_tensor("e3b", [S3, R1], F32)
    x1 = nc.dram_tensor("x1", [N1, R1], F32, addr_space="Shared")
    x2 = nc.dram_tensor("x2", [N2, R1], F32, addr_space="Shared")
    x3 = nc.dram_tensor("x3", [N3, R1], F32, addr_space="Shared")
    gb = nc.dram_tensor("gb", [R, GW], F32)
    gall = nc.dram_tensor("gall", [N, GW], F32, addr_space="Shared")

    rgroups = [list(range(NCORES))]

    with tile.TileContext(nc) as tc:
        with (
            tc.tile_pool(name="const", bufs=1) as constp,
            tc.tile_pool(name="feat", bufs=1) as featp,
            tc.tile_pool(name="arhs", bufs=3) as arhsp,
            tc.tile_pool(name="small", bufs=1) as smallp,
            tc.tile_pool(name="gath", bufs=1) as gathp,
            tc.tile_pool(name="sc", bufs=1) as scp,
            tc.tile_pool(name="psY", bufs=4, space="PSUM") as psY,
            tc.tile_pool(name="psA", bufs=2, space="PSUM") as psA,
            tc.tile_pool(name="psB", bufs=2, space="PSUM") as psB,
        ):
          def _phases():
            ident = constp.tile([128, 128], F32)
            masks.make_identity(nc, ident[:])

            def cload(name, shape):
                t = constp.tile(shape, F32, tag=name)
                nc.sync.dma_start(t[:], globals_map[name][tuple(slice(None) for _ in shape)])
                return t

            globals_map = dict(ebt=ebt, wg1=wg1, bg1c=bg1c, wg2=wg2, bg2c=bg2c,
                               b1m=b1m, b2m=b2m, wb2s=wb2s, b3c=b3c)
            ebt_sb = cload("ebt", [R1, 3])
            wg1_sb = cload("wg1", [R1, R2])
            bg1_sb = cload("bg1c", [R2, 1])
            wg2_sb = cload("wg2", [R2, R3])
            bg2_sb = cload("bg2c", [R3, 1])
            b1m_sb = cload("b1m", [R3, R3])
            b2m_sb = cload("b2m", [R3, R3])
            wb2s_sb = cload("wb2s", [R3, 3])
            b3_sb = cload("b3c", [3, 1])

            # encoder weights: [D, 16] -> sbuf [128, D/128, 16]
            enc_w = []
            for nm, wd, D in (("we1", we1, D1), ("we2", we2, D2), ("we3", we3, D3)):
                t = constp.tile([128, D // 128, R1], F32, tag=nm)
                nc.sync.dma_start(t[:], wd.ap().rearrange("(t p) f -> p t f", p=128))
                enc_w.append(t)

            eidx_sb = constp.tile([128, 6, ECP // 16], I16, tag="eidx")
            nc.sync.dma_start(eidx_sb[:], eidx[:, :, :])

            # ---------------- encoders: xcat[16, 1750] = [e1^T | e2^T | e3^T]
            xcat = smallp.tile([R1, R], F32, tag="xcat")
            enc_cfg = [
                (d1t, enc_w[0], 0, D1, S1, 0),
                (d2t, enc_w[1], 1, D2, S2, S1),
                (d3t, enc_w[2], 2, D3, S3, S1 + S2),
            ]
            for fd, w_sb, bcol, D, S, xoff in enc_cfg:
                nkt = D // 128
                ft = featp.tile([128, nkt, S], F32, tag="feat", name=f"feat{bcol}")
                nc.sync.dma_start(ft[:], fd.ap().rearrange("(t p) s -> p t s", p=128))
                for ns in range(0, S, 512):
                    nw = min(512, S - ns)
                    ps = psA.tile([R1, 512], F32, tag="psa")
                    for t in range(nkt):
                        nc.tensor.matmul(
                            ps[:R1, :nw], w_sb[:, t, :], ft[:, t, ns:ns + nw],
                            start=(t == 0), stop=(t == nkt - 1),
                        )
                    nc.scalar.activation(
                        xcat[:, xoff + ns:xoff + ns + nw], ps[:R1, :nw],
                        AF.Tanh, bias=ebt_sb[:, bcol:bcol + 1],
                    )

            # transpose xcat to natural-order bounce buffers
            for src_off, S, bdram in ((0, S1, e1b), (S1, S2, e2b), (S1 + S2, S3, e3b)):
                for c0 in range(0, S, 128):
                    cw = min(128, S - c0)
                    pt = psB.tile([128, 512], F32, tag="psb")
                    nc.tensor.matmul(
                        pt[:cw, :R1], xcat[:R1, src_off + c0:src_off + c0 + cw],
                        ident[:R1, :R1], is_transpose=True,
                    )
                    st = scp.tile([128, R1], F32, tag="tstage")
                    nc.vector.tensor_copy(st[:cw, :], pt[:cw, :R1])
                    nc.sync.dma_start(bdram[c0:c0 + cw, :], st[:cw, :])

            for bdram, xdram in ((e1b, x1), (e2b, x2), (e3b, x3)):
                nc.gpsimd.collective_compute(
                    "AllGather", ALU.bypass, replica_groups=rgroups,
                    ins=[bdram[:, :]], outs=[xdram[:, :]],
                )

            # load full X (in A-column order) into SBUF: [128, 110, 16]
            xall = smallp.tile([128, len(KT), R1], F32, tag="xall")

            def xsrc(g):
                if g < N1:
                    return x1, g, N1
                if g < N1 + N2:
                    return x2, g - N1, N1 + N2
                return x3, g - N1 - N2, N

            for ti, (t0, tk) in enumerate(KT):
                g = t0
                while g < t0 + tk:
                    dram, loc, lim = xsrc(g)
                    seg = min(t0 + tk, lim) - g
                    nc.sync.dma_start(
                        xall[g - t0:g - t0 + seg, ti, :], dram[loc:loc + seg, :]
                    )
                    g += seg

            if dbg:
                nc.sync.dma_start(dbg_x[:, :], xall[:].rearrange("p t f -> p (t f)"))
            if stage < 2:
                return
            # ---------------- main A@X: Y^T[16, 1750], PSUM-accumulated
            adt = BF16 if BF16_A else F32
            if BF16_A:
                xmm = smallp.tile([128, len(KT), R1], BF16, tag="xbf")
                nc.vector.tensor_copy(xmm[:], xall[:])
            else:
                xmm = xall
            psy = [psY.tile([R1, 512], F32, tag="psy", name=f"psy{i}")
                   for i in range(len(NB))]
            for ti, (t0, tk) in enumerate(KT):
                rt = arhsp.tile([128, R], adt, tag="arhs")
                nc.sync.dma_start(rt[:tk, :], at[t0:t0 + tk, :])
                for nbi, (ns, nw) in enumerate(NB):
                    nc.tensor.matmul(
                        psy[nbi][:R1, :nw], xmm[:tk, ti, :], rt[:tk, ns:ns + nw],
                        start=(ti == 0), stop=(ti == len(KT) - 1),
                    )
            ysb = smallp.tile([R1, R], F32, tag="ysb")
            for nbi, (ns, nw) in enumerate(NB):
                nc.scalar.copy(ysb[:, ns:ns + nw], psy[nbi][:R1, :nw])
            if dbg:
                nc.sync.dma_start(dbg_y[:, :], ysb[:])

            if stage < 3:
                return
            # ---------------- MLP + gather-table build (all transposed)
            hsb = smallp.tile([R2, R], F32, tag="hsb")
            for ns, nw in NB:
                ph = psB.tile([R2, 512], F32, tag="psb")
                nc.tensor.matmul(ph[:R2, :nw], wg1_sb[:R1, :R2], ysb[:R1, ns:ns + nw],
                                 start=True, stop=True)
                nc.scalar.activation(hsb[:R2, ns:ns + nw], ph[:R2, :nw], AF.Tanh,
                                     bias=bg1_sb[:, 0:1])
            # table bands at 32-aligned partition starts (compute-engine APs
            # must start at partition 0/32/64/96): emb@0, T1@32, T2@64, TW@96
            S_sb = smallp.tile([128, R], F32, tag="stab")
            for ns, nw in NB:
                pe = psB.tile([R3, 512], F32, tag="psb")
                nc.tensor.matmul(pe[:R3, :nw], wg2_sb[:R2, :R3], hsb[:R2, ns:ns + nw],
                                 start=True, stop=True)
                nc.scalar.activation(S_sb[0:R3, ns:ns + nw], pe[:R3, :nw], AF.Identity,
                                     bias=bg2_sb[:, 0:1])
            if dbg:
                nc.sync.dma_start(dbg_emb[:, :], S_sb[0:R3, :])
            for ns, nw in NB:
                p1 = psB.tile([R3, 512], F32, tag="psb")
                nc.tensor.matmul(p1[:R3, :nw], b1m_sb[:R3, :R3], S_sb[0:R3, ns:ns + nw],
                                 start=True, stop=True)
                nc.scalar.copy(S_sb[32:48, ns:ns + nw], p1[:R3, :nw])
                p2 = psB.tile([R3, 512], F32, tag="psb")
                nc.tensor.matmul(p2[:R3, :nw], b2m_sb[:R3, :R3], S_sb[0:R3, ns:ns + nw],
                                 start=True, stop=True)
                nc.scalar.copy(S_sb[64:80, ns:ns + nw], p2[:R3, :nw])
                pw = psB.tile([3, 512], F32, tag="psb")
                nc.tensor.matmul(pw[:3, :nw], wb2s_sb[:R3, :3], S_sb[0:R3, ns:ns + nw],
                                 start=True, stop=True)
                nc.scalar.activation(S_sb[96:99, ns:ns + nw], pw[:3, :nw], AF.Identity,
                                     bias=b3_sb[:, 0:1])

            # transpose S -> compact 64-col rows -> gb [1750, 64] -> AllGather
            # (cols 51:64 of gb are unwritten garbage; never read in compute)
            for c0 in range(0, R, 128):
                cw = min(128, R - c0)
                pg = psB.tile([128, 512], F32, tag="psb")
                nc.tensor.matmul(pg[:cw, :128], S_sb[:, c0:c0 + cw],
                                 ident[:, :128], is_transpose=True)
                sg = scp.tile([128, GW], F32, tag="gstage")
                nc.vector.tensor_copy(
                    sg[:cw, :].rearrange("p (g c) -> p g c", c=16),
                    pg[:cw, 0:128].rearrange("p (g c) -> p g c", c=32)[:, :, 0:16],
                )
                nc.sync.dma_start(gb[c0:c0 + cw, :], sg[:cw, :])
            nc.gpsimd.collective_compute(
                "AllGather", ALU.bypass, replica_groups=rgroups,
                ins=[gb[:, :]], outs=[gall[:, :]],
            )
            if dbg:
                nc.sync.dma_start(dbg_g[:, :], gb[:, :])

            if stage < 4:
                return
            # ---------------- edge scoring
            if stage == 35:
                import os
                gch = int(os.environ.get("K_GCHUNK", str(ECP)))
                gd0 = gathp.tile([128, GRP, GW], F32, tag="gd")
                for c0 in range(0, ECP, gch):
                    cn = min(gch, ECP - c0)
                    nc.gpsimd.dma_gather(
                        gd0[:, c0 // 128:(c0 + cn) // 128, :], gall[:, :],
                        eidx_sb[:, 0, c0 // 16:(c0 + cn) // 16],
                        num_idxs=cn, num_idxs_reg=cn, elem_size=GW,
                    )
                if dbg:
                    nc.sync.dma_start(dbg_gd[:, :, :], gd0[:])
                return
            tsb = smallp.tile([128, 6, GRP], F32, tag="tsb")
            for pol in range(2):
                gd = gathp.tile([128, GRP, GW], F32, tag="gd")
                gi = gathp.tile([128, GRP, GW], F32, tag="gi")
                ga = gathp.tile([128, GRP, GW], F32, tag="ga")
                for t, j in ((gd, 3 * pol), (gi, 3 * pol + 1), (ga, 3 * pol + 2)):
                    for c0 in range(0, ECP, 1024):
                        cn = min(1024, ECP - c0)
                        nc.gpsimd.dma_gather(
                            t[:, c0 // 128:(c0 + cn) // 128, :], gall[:, :],
                            eidx_sb[:, j, c0 // 16:(c0 + cn) // 16],
                            num_idxs=cn, num_idxs_reg=cn, elem_size=GW,
                        )
                prod = scp.tile([128, GRP, R3], F32, tag="prod")
                b1 = scp.tile([128, GRP], F32, tag="b1")
                nc.vector.tensor_tensor(prod[:], gd[:, :, 16:32], gi[:, :, 0:16], op=ALU.mult)
                nc.vector.tensor_reduce(b1[:], prod[:], axis=AX.X, op=ALU.add)
                prod2 = scp.tile([128, GRP, R3], F32, tag="prod2")
                b2 = scp.tile([128, GRP], F32, tag="b2")
                nc.vector.tensor_tensor(prod2[:], gd[:, :, 32:48], ga[:, :, 0:16], op=ALU.mult)
                nc.vector.tensor_reduce(b2[:], prod2[:], axis=AX.X, op=ALU.add)
                vt = scp.tile([128, GRP, 3], F32, tag="vt")
                v = scp.tile([128, GRP, 3], F32, tag="v")
                nc.vector.tensor_tensor(vt[:], gd[:, :, 48:51], gi[:, :, 48:51], op=ALU.add)
                nc.vector.tensor_tensor(v[:], vt[:], ga[:, :, 48:51], op=ALU.add)
                a1 = scp.tile([128, GRP], F32, tag="a1")
                a2 = scp.tile([128, GRP], F32, tag="a2")
                nc.vector.tensor_tensor(a1[:], b1[:], v[:, :, 0], op=ALU.add)
                nc.vector.tensor_tensor(a2[:], b2[:], v[:, :, 1], op=ALU.add)
                nc.scalar.activation(tsb[:, 3 * pol + 0, :], a1[:], AF.Tanh)
                nc.scalar.activation(tsb[:, 3 * pol + 1, :], a2[:], AF.Tanh)
                nc.scalar.activation(tsb[:, 3 * pol + 2, :], v[:, :, 2], AF.Tanh)
            nc.sync.dma_start(tout[:, :, :], tsb[:])

          _phases()

    nc.compile()
    _CACHE[key] = nc
    return nc


def _wrap_idx(ids):
    """dma_gather index layout: [128, n/16] int16, 16-partition wrap x8 replicas."""
    assert ids.shape[0] == ECP
    w = ids.astype(np.int16).reshape(ECP // 16, 16).T  # [16, n/16]
    return np.tile(w, (8, 1)).copy()


def _prep_inputs(inputs):
    A = np.asarray(inputs["A"], np.float32)
    d1, d2, d3 = (np.asarray(inputs[k], np.float32) for k in ("d1_fea", "d2_fea", "d3_fea"))
    f32 = lambda k: np.ascontiguousarray(np.asarray(inputs[k], np.float32))
    shared = {
        "we1": f32("W_e1"), "we2": f32("W_e2"), "we3": f32("W_e3"),
        "ebt": np.stack([f32("b_e1"), f32("b_e2"), f32("b_e3")], axis=1),
        "wg1": f32("Wg1"), "bg1c": f32("bg1")[:, None],
        "wg2": f32("Wg2"), "bg2c": f32("bg2")[:, None],
        "b1m": f32("B1"), "b2m": f32("B2m"),
        "wb2s": f32("W_B2") / np.float32(3.0),
        "b3c": ((f32("b_B2") + f32("b_lin")) / np.float32(3.0))[:, None],
    }
    pos = np.asarray(inputs["pos_edges"])
    neg = np.asarray(inputs["neg_edges"])
    offs = np.array([0, N1, 6000], np.int32)  # drug, indi, adr(bugged d3_eb slice)
    in_maps = []
    for c in range(NCORES):
        m = dict(shared)
        r0 = c * R
        m["at"] = np.ascontiguousarray(A[r0:r0 + R, :].T)
        if BF16_A:
            m["at"] = m["at"].astype(ml_dtypes.bfloat16)
        m["d1t"] = np.ascontiguousarray(d1[c * S1:(c + 1) * S1].T)
        m["d2t"] = np.ascontiguousarray(d2[c * S2:(c + 1) * S2].T)
        m["d3t"] = np.ascontiguousarray(d3[c * S3:(c + 1) * S3].T)
        eidx = np.zeros((128, 6, ECP // 16), np.int16)
        for pol, edges in enumerate((pos, neg)):
            sl = edges[c * EC:(c + 1) * EC]
            for role in range(3):
                ids = np.zeros(ECP, np.int32)
                ids[:EC] = sl[:, role, 1].astype(np.int32) + offs[role]
                eidx[:, 3 * pol + role, :] = _wrap_idx(ids)
        m["eidx"] = eidx
        in_maps.append(m)
    return in_maps


def _finish(results, inputs):
    wsim = np.asarray(inputs["W_sim"], np.float32)[:, 0]
    bsim = np.asarray(inputs["b_sim"], np.float32)[0]
    parts = []
    for c in range(NCORES):
        arr = results[c]["tout"]  # [128, 6, 98]; edge g*128+p at [p, j, g]
        parts.append(arr.transpose(1, 2, 0).reshape(6, ECP)[:, :EC])
    T = np.concatenate(parts, axis=1).astype(np.float32)  # [6, 100000]
    Se = wsim[0] * T[0] + wsim[1] * T[1] + wsim[2] * T[2] + bsim
    Se0 = wsim[0] * T[3] + wsim[1] * T[4] + wsim[2] * T[5] + bsim
    m0 = np.float32(Se0.mean())
    loss = np.log1p(np.exp(m0 - Se)).mean()
    return np.asarray(loss, dtype=np.float32)


def run(inputs, trace=False, dbg=False):
    nc = _build(dbg=dbg)
    in_maps = _prep_inputs(inputs)
    res = run_bass_kernel_spmd(nc, in_maps, list(range(NCORES)), trace=trace)
    return res


def kernel(**inputs) -> np.ndarray:
    res = run(inputs)
    return _finish(res.results, inputs)
